# revision 13
# baseline (speedup 1.0000x reference)
"""Trainium2 Bass kernel for nn_AttentionLayer (B=8, S=1024, D=1024, H=16, HD=64).

Strategy: pure data parallelism — one batch element per NeuronCore (8 cores).
Weights are replicated (pre-transposed on host so the contraction dim lands on
SBUF partitions); x is sharded on batch and pre-transposed per shard.

Per-core compute layout (all transposes eliminated by construction):
  Qt/Kt [dout, s]  = W^T-stacked @ x^T        (d on partitions)
  V     [s, dout]  natural                     (s on partitions)
  scoresT[k, q]    = Kt_h^T @ Qt_h             (k on partitions, q free)
  expT   [k, q]    = exp(scale*scoresT + pad_bias_k)  (pad mask = per-partition
                     bias fused into the Exp activation; causal handled by
                     column skipping + one affine_select near the diagonal)
  avT -> out[q, d] via lhsT=[V_h | 1] (ones column also produces the softmax
                     denominator as psum row 64)
  attn_outT [dmid, s] assembled via SBUF->SBUF DMA, normalized by 1/denom
                     (broadcast via a small DRAM bounce), then
  out [s, dout]    = attn_outT^T @ Wo^T + bo   (bias via K=1 matmul)

Matmul dtype: float32r (full-speed fp32 on the PE at N>=256). PSUM-evacuation
copies double as the required f32r rounding producers.
"""

import os
import sys
import types

import numpy as np

B, S, D, H, HD = 8, 1024, 1024, 16, 64
NT = D // 128          # 8 partition tiles
PAD_ID = 1.0
NEG = -1e9
SCALE = 1.0 / 8.0      # 1/sqrt(HD)

# matmul compute dtype: "f32r" | "bf16" | "f32"
# f32r would be full fp32 precision at bf16 PE speed, but hangs TRN2 hardware
# (observed empirically; the compiler's FP32-HI FWL hang-guard does not cover
# float32r). bf16 runs at the same PE rate with half the DMA traffic.
MM_MODE = os.environ.get("KERNEL_MM_MODE", "bf16")

_CACHE = {}
LAST_RESULT = None
LAST_EXEC_NS = None


def _install_trace_hook():
    """Provide antenv.axon_hooks (missing in this image) so trace=True works."""
    try:
        import antenv
        if "antenv.axon_hooks" in sys.modules:
            return True
        m = types.ModuleType("antenv.axon_hooks")
        _hook = [None]
        m.set_axon_ntff_profile_hook = lambda h: _hook.__setitem__(0, h)
        m.get_axon_ntff_profile_hook = lambda: _hook[0]
        sys.modules["antenv.axon_hooks"] = m
        antenv.axon_hooks = m
        from trn_agent_boot.trn_boot import _ntff_profile_via_ctypes
        hook = _ntff_profile_via_ctypes("/opt/axon/libaxon_pjrt.so")
        if hook is None:
            return False
        m.set_axon_ntff_profile_hook(hook)
        return True
    except Exception:
        return False


def _build_graph():
    import concourse.bass as bass
    import concourse.mybir as mybir
    import concourse.tile as tile
    from concourse import bacc

    F32 = mybir.dt.float32
    if MM_MODE == "f32r":
        MMD = mybir.dt.float32r
    elif MM_MODE == "bf16":
        MMD = mybir.dt.bfloat16
    else:
        MMD = mybir.dt.float32
    AluOp = mybir.AluOpType
    Act = mybir.ActivationFunctionType

    nc = bacc.Bacc(target_bir_lowering=False)

    xT = nc.declare_dram_parameter("xT", [D, S], MMD, isOutput=False)
    WqT = nc.declare_dram_parameter("WqT", [D, D], MMD, isOutput=False)
    WkT = nc.declare_dram_parameter("WkT", [D, D], MMD, isOutput=False)
    WvT = nc.declare_dram_parameter("WvT", [D, D], MMD, isOutput=False)
    WoT = nc.declare_dram_parameter("WoT", [D, D], MMD, isOutput=False)
    bq = nc.declare_dram_parameter("bq", [D], F32, isOutput=False)
    bk = nc.declare_dram_parameter("bk", [D], F32, isOutput=False)
    bv = nc.declare_dram_parameter("bv", [D], MMD, isOutput=False)
    bo = nc.declare_dram_parameter("bo", [D], MMD, isOutput=False)
    ones_p = nc.declare_dram_parameter("ones", [S], MMD, isOutput=False)
    ids = nc.declare_dram_parameter("ids", [S], F32, isOutput=False)
    out_e = nc.declare_dram_parameter("out", [S, D], F32, isOutput=True)

    denom_d = nc.dram_tensor("denom_d", [H, S], MMD)
    recip_d = nc.dram_tensor("recip_d", [H, S], MMD)

    def dep(later, earlier, reason):
        bass._add_dep_helper(later.ins, earlier.ins, reason=reason)

    with tile.TileContext(nc) as tc:
        with tc.tile_pool(name="const", bufs=1) as cp, \
             tc.tile_pool(name="qkv", bufs=1) as qp:

            # ---- constants ----
            ids_sb = cp.tile([128, NT], F32, tag="ids", name="ids_sb")
            nc.sync.dma_start(out=ids_sb[:], in_=ids[:].rearrange("(c p) -> p c", p=128))
            pad_cols = cp.tile([128, NT], F32, tag="pad", name="pad_cols")
            nc.vector.tensor_scalar(out=pad_cols[:], in0=ids_sb[:],
                                    scalar1=PAD_ID, scalar2=NEG,
                                    op0=AluOp.is_equal, op1=AluOp.mult)
            bq_col = cp.tile([128, NT], F32, tag="bqc", name="bq_col")
            nc.sync.dma_start(out=bq_col[:], in_=bq[:].rearrange("(t p) -> p t", p=128))
            bk_col = cp.tile([128, NT], F32, tag="bkc", name="bk_col")
            nc.sync.dma_start(out=bk_col[:], in_=bk[:].rearrange("(t p) -> p t", p=128))
            bv_row = cp.tile([1, D], MMD, tag="bvr", name="bv_row")
            nc.sync.dma_start(out=bv_row[:], in_=bv[None, :])
            bo_row = cp.tile([1, D], MMD, tag="bor", name="bo_row")
            nc.sync.dma_start(out=bo_row[:], in_=bo[None, :])
            ones_row = cp.tile([1, S], MMD, tag="ones", name="ones_row")
            nc.sync.dma_start(out=ones_row[:], in_=ones_p[None, :])

            # ---- persistent per-core tensors ----
            Qt = [qp.tile([128, S], MMD, tag=f"qt{t}", name=f"qt{t}") for t in range(NT)]
            Kt = [qp.tile([128, S], MMD, tag=f"kt{t}", name=f"kt{t}") for t in range(NT)]
            Vx = [qp.tile([128, H * (HD + 1)], MMD, tag=f"vx{t}", name=f"vx{t}") for t in range(NT)]

            # ============ Phase B: projections ============
            with tc.tile_pool(name="xw", bufs=1) as xp, \
                 tc.tile_pool(name="wst", bufs=8) as wp, \
                 tc.tile_pool(name="psp", bufs=4, space="PSUM") as pp:

                xT_sb = [xp.tile([128, S], MMD, tag=f"x{c}", name=f"x{c}") for c in range(NT)]
                for c in range(NT):
                    nc.sync.dma_start(out=xT_sb[c][:], in_=xT[c * 128:(c + 1) * 128, :])

                def stream_w(w_ext):
                    tiles = []
                    for c in range(NT):
                        t = wp.tile([128, D], MMD, tag="w", name="w_t")
                        nc.sync.dma_start(out=t[:], in_=w_ext[c * 128:(c + 1) * 128, :])
                        tiles.append(t)
                    return tiles

                # Qt / Kt: out tile m = sum_c W?T[c][:, m-block]^T @ xT[c]
                for w_ext, dst, bias_col in ((WqT, Qt, bq_col), (WkT, Kt, bk_col)):
                    w_sb = stream_w(w_ext)
                    for mp in range(NT // 2):
                        for mm in range(2):
                            m = mp * 2 + mm
                            for n in range(2):
                                ps = pp.tile([128, 512], F32, tag="pp", name="ps")
                                for c in range(NT):
                                    nc.tensor.matmul(
                                        ps[:],
                                        w_sb[c][:, m * 128:(m + 1) * 128],
                                        xT_sb[c][:, n * 512:(n + 1) * 512],
                                        start=(c == 0), stop=(c == NT - 1))
                                nc.vector.tensor_scalar(
                                    out=dst[m][:, n * 512:(n + 1) * 512], in0=ps[:],
                                    scalar1=bias_col[:, m:m + 1], scalar2=None,
                                    op0=AluOp.add)

                # V: out tile m (s), strided into Vx (65-stride per head), ones col
                w_sb = stream_w(WvT)
                for m in range(NT):
                    vdst = Vx[m][:].rearrange("p (h e) -> p h e", e=HD + 1)
                    nc.sync.dma_start(
                        out=vdst[:, :, HD:HD + 1],
                        in_=ones_p[:H][None, :, None].broadcast_to([128, H, 1]))
                    for n in range(2):
                        ps = pp.tile([128, 512], F32, tag="pp", name="ps")
                        for c in range(NT):
                            nc.tensor.matmul(
                                ps[:],
                                xT_sb[c][:, m * 128:(m + 1) * 128],
                                w_sb[c][:, n * 512:(n + 1) * 512],
                                start=(c == 0), stop=False)
                        nc.tensor.matmul(ps[:], ones_row[:, :128],
                                         bv_row[:, n * 512:(n + 1) * 512],
                                         start=False, stop=True)
                        nc.vector.tensor_copy(
                            out=vdst[:, n * 8:(n + 1) * 8, 0:HD],
                            in_=ps[:].rearrange("p (h e) -> p h e", e=HD))

            # ============ Phase C: attention ============
            with tc.tile_pool(name="aot", bufs=1) as ap_pool:
              aoT = [ap_pool.tile([128, S], MMD, tag=f"ao{t}", name=f"ao{t}")
                     for t in range(NT)]
              with tc.tile_pool(name="expp", bufs=3) as ep, \
                 tc.tile_pool(name="avst", bufs=4) as avs, \
                 tc.tile_pool(name="dn", bufs=2) as dnp, \
                 tc.tile_pool(name="rcp", bufs=2) as rcpp, \
                 tc.tile_pool(name="pssc", bufs=2, space="PSUM") as psc, \
                 tc.tile_pool(name="psav", bufs=1, space="PSUM") as pav:

                denom_writes = {h: [] for h in range(H)}
                recip_writes = {}

                for h in range(H):
                    t, base = h // 2, (h % 2) * 64
                    av_ps = [pav.tile([HD + 1, 256], F32, tag=f"av{g}", name=f"av{g}")
                             for g in range(4)]
                    for c in range(NT):
                        qs = 256 * (c // 2)
                        cols = S - qs
                        sc = psc.tile([128, 1024], F32, tag="sc", name="sc")
                        for n0 in range(qs, S, 512):
                            w = min(512, S - n0)
                            nc.tensor.matmul(
                                sc[:, n0 - qs:n0 - qs + w],
                                Kt[t][base:base + 64, c * 128:(c + 1) * 128],
                                Qt[t][base:base + 64, n0:n0 + w],
                                start=True, stop=True)
                        ex = ep.tile([128, 1024], MMD, tag="ex", name="ex")
                        nc.scalar.activation(out=ex[:, 0:cols], in_=sc[:, 0:cols],
                                             func=Act.Exp,
                                             bias=pad_cols[:, c:c + 1], scale=SCALE)
                        nc.gpsimd.affine_select(
                            out=ex[:, 0:256], in_=ex[:, 0:256],
                            compare_op=AluOp.is_ge, fill=0.0,
                            base=qs - 128 * c, channel_multiplier=-1,
                            pattern=[[1, 256]])
                        for g in range(c // 2, 4):
                            nc.tensor.matmul(
                                av_ps[g][:],
                                Vx[c][:, h * (HD + 1):(h + 1) * (HD + 1)],
                                ex[:, 256 * g - qs:256 * g - qs + 256],
                                start=(c == 0), stop=(c == min(2 * g + 1, NT - 1)))
                    for g in range(4):
                        st = avs.tile([HD + 1, 256], MMD, tag="st", name="st")
                        nc.vector.tensor_copy(out=st[:], in_=av_ps[g][:])
                        nc.sync.dma_start(
                            out=aoT[t][base:base + HD, 256 * g:256 * (g + 1)],
                            in_=st[0:HD, :])
                        dw = nc.sync.dma_start(
                            out=denom_d[h:h + 1, 256 * g:256 * (g + 1)],
                            in_=st[HD:HD + 1, :])
                        denom_writes[h].append(dw)

                    # per-pair denominator -> reciprocal -> broadcast -> normalize
                    if h % 2 == 1:
                        d_sb = dnp.tile([128, 2, NT], MMD, tag="dsb", name="d_sb")
                        rd = nc.sync.dma_start(
                            out=d_sb[:],
                            in_=denom_d[2 * t:2 * t + 2, :].rearrange(
                                "h (g p) -> p h g", p=128))
                        for hh in (2 * t, 2 * t + 1):
                            for w_ in denom_writes[hh]:
                                dep(rd, w_, reason="denom RAW via DRAM")
                        r_sb = dnp.tile([128, 2, NT], MMD, tag="rsb", name="r_sb")
                        with nc.allow_low_precision(
                                reason="softmax recip in compute dtype; "
                                       "error budget verified offline"):
                            nc.vector.reciprocal(out=r_sb[:], in_=d_sb[:])
                        wr = nc.sync.dma_start(
                            out=recip_d[2 * t:2 * t + 2, :].rearrange(
                                "h (g p) -> p h g", p=128),
                            in_=r_sb[:])
                        recip_writes[t] = wr
                        rec = rcpp.tile([128, S], MMD, tag="rec", name="rec")
                        b0 = nc.sync.dma_start(
                            out=rec[0:64, :],
                            in_=recip_d[2 * t, :][None, :].broadcast_to([64, S]))
                        b1 = nc.sync.dma_start(
                            out=rec[64:128, :],
                            in_=recip_d[2 * t + 1, :][None, :].broadcast_to([64, S]))
                        dep(b0, wr, reason="recip RAW via DRAM")
                        dep(b1, wr, reason="recip RAW via DRAM")
                        nc.vector.tensor_mul(aoT[t][:], aoT[t][:], rec[:])

              # ============ Phase E: output projection ============
              with tc.tile_pool(name="wo", bufs=8) as wop, \
                 tc.tile_pool(name="ost", bufs=3) as osp, \
                 tc.tile_pool(name="psf", bufs=4, space="PSUM") as pf:
                wo_sb = []
                for c in range(NT):
                    w_t = wop.tile([128, D], MMD, tag="wo", name="wo_t")
                    nc.sync.dma_start(out=w_t[:], in_=WoT[c * 128:(c + 1) * 128, :])
                    wo_sb.append(w_t)
                for m in range(NT):
                    for n in range(2):
                        ps = pf.tile([128, 512], F32, tag="pf", name="psf")
                        for c in range(NT):
                            nc.tensor.matmul(
                                ps[:],
                                aoT[c][:, m * 128:(m + 1) * 128],
                                wo_sb[c][:, n * 512:(n + 1) * 512],
                                start=(c == 0), stop=False)
                        nc.tensor.matmul(ps[:], ones_row[:, :128],
                                         bo_row[:, n * 512:(n + 1) * 512],
                                         start=False, stop=True)
                        ot = osp.tile([128, 512], F32, tag="ot", name="ot")
                        nc.scalar.copy(out=ot[:], in_=ps[:])
                        nc.sync.dma_start(
                            out=out_e[m * 128:(m + 1) * 128, n * 512:(n + 1) * 512],
                            in_=ot[:])
    nc.finalize()
    return nc


def _np_mm_dtype():
    if MM_MODE == "bf16":
        import ml_dtypes
        return ml_dtypes.bfloat16
    return np.float32


def kernel(x, input_ids, Wq, bq, Wk, bk, Wv, bv, Wo, bo):
    global LAST_RESULT, LAST_EXEC_NS
    from concourse.bass_utils import run_bass_kernel_spmd

    x = np.asarray(x, dtype=np.float32)
    input_ids = np.asarray(input_ids)
    mmdt = _np_mm_dtype()

    if "nc" not in _CACHE:
        _CACHE["nc"] = _build_graph()
    nc = _CACHE["nc"]

    WqT = np.ascontiguousarray(np.asarray(Wq, np.float32).T).astype(mmdt)
    WkT = np.ascontiguousarray(np.asarray(Wk, np.float32).T).astype(mmdt)
    WvT = np.ascontiguousarray(np.asarray(Wv, np.float32).T).astype(mmdt)
    WoT = np.ascontiguousarray(np.asarray(Wo, np.float32).T).astype(mmdt)
    ones = np.ones([S], mmdt)
    shared = {
        "WqT": WqT, "WkT": WkT, "WvT": WvT, "WoT": WoT,
        "bq": np.asarray(bq, np.float32), "bk": np.asarray(bk, np.float32),
        "bv": np.asarray(bv, np.float32).astype(mmdt),
        "bo": np.asarray(bo, np.float32).astype(mmdt),
        "ones": ones,
    }
    in_maps = []
    for b in range(B):
        m = dict(shared)
        m["xT"] = np.ascontiguousarray(x[b].T).astype(mmdt)
        m["ids"] = input_ids[b].astype(np.float32)
        in_maps.append(m)

    trace = os.environ.get("KERNEL_TRACE", "0") == "1" and _install_trace_hook()
    res = run_bass_kernel_spmd(nc, in_maps, core_ids=list(range(B)), trace=trace)
    LAST_RESULT = res
    LAST_EXEC_NS = res.exec_time_ns
    return np.stack([res.results[b]["out"] for b in range(B)]).astype(np.float32)


# revision 18
# speedup vs baseline: 1.3969x; 1.3969x over previous
"""Trainium2 Bass kernel for nn_AttentionLayer (B=8, S=1024, D=1024, H=16, HD=64).

Strategy: pure data parallelism — one batch element per NeuronCore (8 cores).
Weights are replicated (pre-transposed on host so the contraction dim lands on
SBUF partitions); x is sharded on batch and pre-transposed per shard.

Per-core compute layout (all transposes eliminated by construction):
  Qt/Kt [dout, s]  = W^T-stacked @ x^T         (d on partitions)
  Vx    [s, dout]  natural, 65-strided per head with a ones column; padded
                    keys' rows are zeroed (this IS the pad mask: they then
                    contribute 0 to both attention output and denominator)
  scoresT[k, q]    = Kt_h^T @ Qt_h             (k on partitions, q free);
                    even/odd heads sit at partition bases 0/64, so the two
                    K=64 matmuls of a head pair run concurrently on the PE
                    (disjoint row groups). Causal masking of the diagonal
                    256-col window = identity-matmul accumulating a -1e9
                    triangle mask into the scores PSUM.
  expT   [k, q]    = exp(scoresT / 8)          (ACT, PSUM->SBUF, bf16 out)
  avT -> out[q, d] via lhsT=[V_h | 1]: ones column also produces the softmax
                    denominator as psum row 64; accumulated per 512-wide
                    q-chunk so a head pair needs 4 PSUM banks.
  attn_outT [dmid, s] assembled via SBUF->SBUF DMA, normalized by 1/denom
                    (per-pair DRAM bounce + partition-broadcast DMA), then
  out [s, dout]    = attn_outT^T @ Wo^T + bo   (bias via K=1 matmul)

Matmul dtype: bf16 (f32r would be full fp32 precision at the same PE rate,
but hangs TRN2 hardware - observed empirically). End-to-end rel err vs the
fp32 reference is ~4e-3.
"""

import os
import sys
import types

import numpy as np

B, S, D, H, HD = 8, 1024, 1024, 16, 64
NT = D // 128          # 8 partition tiles
PAD_ID = 1.0
NEG = -1e9
SCALE = 1.0 / 8.0      # 1/sqrt(HD)

MM_MODE = os.environ.get("KERNEL_MM_MODE", "bf16")

_CACHE = {}
LAST_RESULT = None
LAST_EXEC_NS = None


def _install_trace_hook():
    """Provide antenv.axon_hooks (missing in this image) so trace=True works."""
    try:
        import antenv
        if "antenv.axon_hooks" in sys.modules:
            return True
        m = types.ModuleType("antenv.axon_hooks")
        _hook = [None]
        m.set_axon_ntff_profile_hook = lambda h: _hook.__setitem__(0, h)
        m.get_axon_ntff_profile_hook = lambda: _hook[0]
        sys.modules["antenv.axon_hooks"] = m
        antenv.axon_hooks = m
        from trn_agent_boot.trn_boot import _ntff_profile_via_ctypes
        hook = _ntff_profile_via_ctypes("/opt/axon/libaxon_pjrt.so")
        if hook is None:
            return False
        m.set_axon_ntff_profile_hook(hook)
        return True
    except Exception:
        return False


def _build_graph():
    import concourse.bass as bass
    import concourse.mybir as mybir
    import concourse.tile as tile
    from concourse import bacc

    F32 = mybir.dt.float32
    MMD = {"bf16": mybir.dt.bfloat16, "f32r": mybir.dt.float32r,
           "f32": mybir.dt.float32}[MM_MODE]
    AluOp = mybir.AluOpType
    Act = mybir.ActivationFunctionType

    nc = bacc.Bacc(target_bir_lowering=False)

    xT = nc.declare_dram_parameter("xT", [D, S], MMD, isOutput=False)
    WqT = nc.declare_dram_parameter("WqT", [D, D], MMD, isOutput=False)
    WkT = nc.declare_dram_parameter("WkT", [D, D], MMD, isOutput=False)
    WvT = nc.declare_dram_parameter("WvT", [D, D], MMD, isOutput=False)
    WoT = nc.declare_dram_parameter("WoT", [D, D], MMD, isOutput=False)
    bv = nc.declare_dram_parameter("bv", [D], MMD, isOutput=False)
    bo = nc.declare_dram_parameter("bo", [D], MMD, isOutput=False)
    ones_p = nc.declare_dram_parameter("ones", [S], MMD, isOutput=False)
    # smalls: [128, 24] f32 = ids_r | bq_r | bk_r (each [128, 8], host-packed)
    smalls = nc.declare_dram_parameter("smalls", [128, 3 * NT], F32, isOutput=False)
    # causal masks [128, 512] = maskA (even chunks) | maskB (odd chunks)
    masks_p = nc.declare_dram_parameter("masks", [128, 512], MMD, isOutput=False)
    ident_p = nc.declare_dram_parameter("ident", [128, 128], MMD, isOutput=False)
    out_e = nc.declare_dram_parameter("out", [S, D], F32, isOutput=True)

    denom_d = nc.dram_tensor("denom_d", [H, S], MMD)
    recip_d = nc.dram_tensor("recip_d", [H, S], MMD)

    def dep(later, earlier, reason):
        bass._add_dep_helper(later.ins, earlier.ins, reason=reason)

    with tile.TileContext(nc) as tc:
        with tc.tile_pool(name="const", bufs=1) as cp, \
             tc.tile_pool(name="qkv", bufs=1) as qp:

            # ---- constants ----
            sm = cp.tile([128, 3 * NT], F32, tag="sm", name="sm")
            nc.sync.dma_start(out=sm[:], in_=smalls[:])
            pad01 = cp.tile([128, NT], F32, tag="pad01", name="pad01")
            nc.vector.tensor_scalar(out=pad01[:], in0=sm[:, 0:NT],
                                    scalar1=PAD_ID, scalar2=None,
                                    op0=AluOp.not_equal)
            bq_col = sm[:, NT:2 * NT]
            bk_col = sm[:, 2 * NT:3 * NT]
            bv_row = cp.tile([1, D], MMD, tag="bvr", name="bv_row")
            nc.sync.dma_start(out=bv_row[:], in_=bv[None, :])
            bo_row = cp.tile([1, D], MMD, tag="bor", name="bo_row")
            nc.sync.dma_start(out=bo_row[:], in_=bo[None, :])
            ones_row = cp.tile([1, S], MMD, tag="ones", name="ones_row")
            nc.sync.dma_start(out=ones_row[:], in_=ones_p[None, :])
            masks_sb = cp.tile([128, 512], MMD, tag="masks", name="masks_sb")
            nc.sync.dma_start(out=masks_sb[:], in_=masks_p[:])
            ident = cp.tile([128, 128], MMD, tag="ident", name="ident")
            nc.sync.dma_start(out=ident[:], in_=ident_p[:])

            # ---- persistent per-core tensors ----
            Qt = [qp.tile([128, S], MMD, tag=f"qt{t}", name=f"qt{t}")
                  for t in range(NT)]
            Kt = [qp.tile([128, S], MMD, tag=f"kt{t}", name=f"kt{t}")
                  for t in range(NT)]
            Vx = [qp.tile([128, H * (HD + 1)], MMD, tag=f"vx{t}", name=f"vx{t}")
                  for t in range(NT)]

            # ============ Phase B: projections ============
            with tc.tile_pool(name="xw", bufs=1) as xp, \
                 tc.tile_pool(name="wst", bufs=8) as wp, \
                 tc.tile_pool(name="psp", bufs=4, space="PSUM") as pp:

                xT_sb = [xp.tile([128, S], MMD, tag=f"x{c}", name=f"x{c}")
                         for c in range(NT)]
                for c in range(NT):
                    nc.sync.dma_start(out=xT_sb[c][:], in_=xT[c * 128:(c + 1) * 128, :])

                def stream_w(w_ext):
                    tiles = []
                    for c in range(NT):
                        t = wp.tile([128, D], MMD, tag="w", name="w_t")
                        nc.sync.dma_start(out=t[:], in_=w_ext[c * 128:(c + 1) * 128, :])
                        tiles.append(t)
                    return tiles

                for w_ext, dst, bias_col in ((WqT, Qt, bq_col), (WkT, Kt, bk_col)):
                    w_sb = stream_w(w_ext)
                    for m in range(NT):
                        for n in range(2):
                            ps = pp.tile([128, 512], F32, tag="pp", name="ps")
                            for c in range(NT):
                                nc.tensor.matmul(
                                    ps[:],
                                    w_sb[c][:, m * 128:(m + 1) * 128],
                                    xT_sb[c][:, n * 512:(n + 1) * 512],
                                    start=(c == 0), stop=(c == NT - 1))
                            nc.vector.tensor_scalar(
                                out=dst[m][:, n * 512:(n + 1) * 512], in0=ps[:],
                                scalar1=bias_col[:, m:m + 1], scalar2=None,
                                op0=AluOp.add)

                w_sb = stream_w(WvT)
                for m in range(NT):
                    vdst = Vx[m][:].rearrange("p (h e) -> p h e", e=HD + 1)
                    nc.vector.memset(vdst[:, :, HD:HD + 1], 1.0)
                    for n in range(2):
                        ps = pp.tile([128, 512], F32, tag="pp", name="ps")
                        for c in range(NT):
                            nc.tensor.matmul(
                                ps[:],
                                xT_sb[c][:, m * 128:(m + 1) * 128],
                                w_sb[c][:, n * 512:(n + 1) * 512],
                                start=(c == 0), stop=False)
                        nc.tensor.matmul(ps[:], ones_row[:, :128],
                                         bv_row[:, n * 512:(n + 1) * 512],
                                         start=False, stop=True)
                        nc.vector.tensor_copy(
                            out=vdst[:, n * 8:(n + 1) * 8, 0:HD],
                            in_=ps[:].rearrange("p (h e) -> p h e", e=HD))
                    # pad mask: zero whole rows (keys) where ids == PAD,
                    # including the ones column -> denominator excludes them
                    nc.vector.tensor_scalar(
                        out=Vx[m][:], in0=Vx[m][:],
                        scalar1=pad01[:, m:m + 1], scalar2=None,
                        op0=AluOp.mult)

            # ============ Phase C: attention (head pairs) ============
            with tc.tile_pool(name="aot", bufs=1) as ap_pool, \
                 tc.tile_pool(name="wo", bufs=8) as wop:
              aoT = [ap_pool.tile([128, S], MMD, tag=f"ao{t}", name=f"ao{t}")
                     for t in range(NT)]
              with tc.tile_pool(name="expp", bufs=4) as ep, \
                   tc.tile_pool(name="avst", bufs=2) as avs, \
                   tc.tile_pool(name="dn", bufs=2) as dnp, \
                   tc.tile_pool(name="rcp", bufs=2) as rcpp, \
                   tc.tile_pool(name="pssc", bufs=2, space="PSUM") as psc, \
                   tc.tile_pool(name="psav", bufs=1, space="PSUM") as pav:

                # prefetch Wo during attention
                wo_sb = []
                for c in range(NT):
                    w_t = wop.tile([128, D], MMD, tag="wo", name="wo_t")
                    nc.sync.dma_start(out=w_t[:], in_=WoT[c * 128:(c + 1) * 128, :])
                    wo_sb.append(w_t)

                denom_writes = {}
                for t in range(NT):         # head pair (2t, 2t+1)
                    av_ps = {(par, g): pav.tile([HD + 1, 512], F32,
                                                tag=f"av{par}{g}",
                                                name=f"av{par}{g}")
                             for par in range(2) for g in range(2)}
                    ex_t = {}
                    for c in range(NT):
                        qs = 256 * (c // 2)
                        cols = S - qs
                        for par in range(2):
                            base = par * 64
                            sc = psc.tile([128, 1024], F32, tag="sc", name="sc")
                            nchunks = list(range(qs, S, 512))
                            for n0 in nchunks:
                                w = min(512, S - n0)
                                nc.tensor.matmul(
                                    sc[:, n0 - qs:n0 - qs + w],
                                    Kt[t][base:base + 64, c * 128:(c + 1) * 128],
                                    Qt[t][base:base + 64, n0:n0 + w],
                                    start=True, stop=not (n0 == qs))
                            # causal mask for the diagonal window via
                            # identity-matmul accumulation (bank A group)
                            nc.tensor.matmul(
                                sc[:, 0:256], ident[:],
                                masks_sb[:, 256 * (c % 2):256 * (c % 2) + 256],
                                start=False, stop=True)
                            ex = ep.tile([128, 1024], MMD, tag="ex",
                                         name="ex")
                            ex_t[par] = ex
                            if qs % 512:
                                # zero the gap so 512-aligned avT reads are valid
                                nc.vector.memset(ex[:, qs - 256:qs], 0.0)
                            nc.scalar.activation(out=ex[:, qs:S],
                                                 in_=sc[:, 0:cols],
                                                 func=Act.Exp, scale=SCALE)
                            for g in range(2):
                                if c <= 4 * g + 3:
                                    nc.tensor.matmul(
                                        av_ps[(par, g)][:],
                                        Vx[c][:, (2 * t + par) * (HD + 1):
                                              (2 * t + par + 1) * (HD + 1)],
                                        ex[:, 512 * g:512 * (g + 1)],
                                        start=(c == 0),
                                        stop=(c == min(4 * g + 3, NT - 1)))
                    for par in range(2):
                        h = 2 * t + par
                        st = avs.tile([HD + 1, S], MMD, tag="st", name="st")
                        for g in range(2):
                            nc.vector.tensor_copy(
                                out=st[:, 512 * g:512 * (g + 1)],
                                in_=av_ps[(par, g)][:])
                        nc.sync.dma_start(
                            out=aoT[t][par * 64:par * 64 + HD, :],
                            in_=st[0:HD, :])
                        dw = nc.sync.dma_start(
                            out=denom_d[h:h + 1, :], in_=st[HD:HD + 1, :])
                        denom_writes[h] = dw

                    # per-pair denominator -> reciprocal -> broadcast -> norm
                    d_sb = dnp.tile([128, 2, NT], MMD, tag="dsb", name="d_sb")
                    rd = nc.sync.dma_start(
                        out=d_sb[:],
                        in_=denom_d[2 * t:2 * t + 2, :].rearrange(
                            "h (g p) -> p h g", p=128))
                    dep(rd, denom_writes[2 * t], reason="denom RAW via DRAM")
                    dep(rd, denom_writes[2 * t + 1], reason="denom RAW via DRAM")
                    r_sb = dnp.tile([128, 2, NT], MMD, tag="rsb", name="r_sb")
                    with nc.allow_low_precision(
                            reason="softmax recip in compute dtype; "
                                   "error budget verified offline"):
                        nc.vector.reciprocal(out=r_sb[:], in_=d_sb[:])
                    wr = nc.sync.dma_start(
                        out=recip_d[2 * t:2 * t + 2, :].rearrange(
                            "h (g p) -> p h g", p=128),
                        in_=r_sb[:])
                    rec = rcpp.tile([128, S], MMD, tag="rec", name="rec")
                    b0 = nc.sync.dma_start(
                        out=rec[0:64, :],
                        in_=recip_d[2 * t, :][None, :].broadcast_to([64, S]))
                    b1 = nc.sync.dma_start(
                        out=rec[64:128, :],
                        in_=recip_d[2 * t + 1, :][None, :].broadcast_to([64, S]))
                    dep(b0, wr, reason="recip RAW via DRAM")
                    dep(b1, wr, reason="recip RAW via DRAM")
                    nc.vector.tensor_mul(aoT[t][:], aoT[t][:], rec[:])

              # ============ Phase E: output projection ============
              with tc.tile_pool(name="ost", bufs=3) as osp, \
                   tc.tile_pool(name="psf", bufs=4, space="PSUM") as pf:
                for m in range(NT):
                    for n in range(2):
                        ps = pf.tile([128, 512], F32, tag="pf", name="psf")
                        for c in range(NT):
                            nc.tensor.matmul(
                                ps[:],
                                aoT[c][:, m * 128:(m + 1) * 128],
                                wo_sb[c][:, n * 512:(n + 1) * 512],
                                start=(c == 0), stop=False)
                        nc.tensor.matmul(ps[:], ones_row[:, :128],
                                         bo_row[:, n * 512:(n + 1) * 512],
                                         start=False, stop=True)
                        ot = osp.tile([128, 512], F32, tag="ot", name="ot")
                        nc.scalar.copy(out=ot[:], in_=ps[:])
                        nc.sync.dma_start(
                            out=out_e[m * 128:(m + 1) * 128, n * 512:(n + 1) * 512],
                            in_=ot[:])
    nc.finalize()
    return nc


def _np_mm_dtype():
    if MM_MODE == "bf16":
        import ml_dtypes
        return ml_dtypes.bfloat16
    return np.float32


def _host_consts(mmdt):
    jj = np.arange(256)[None, :]
    pp = np.arange(128)[:, None]
    maskA = np.where((jj < 128) & (jj < pp), NEG, 0.0)
    maskB = np.where((jj < 128) | (jj - 128 < pp), NEG, 0.0)
    masks = np.concatenate([maskA, maskB], axis=1).astype(mmdt)
    ident = np.eye(128, dtype=np.float32).astype(mmdt)
    return masks, ident


def build_in_maps(x, input_ids, Wq, bq, Wk, bk, Wv, bv, Wo, bo):
    x = np.asarray(x, dtype=np.float32)
    input_ids = np.asarray(input_ids)
    mmdt = _np_mm_dtype()
    masks, ident = _host_consts(mmdt)
    bq_r = np.ascontiguousarray(np.asarray(bq, np.float32).reshape(NT, 128).T)
    bk_r = np.ascontiguousarray(np.asarray(bk, np.float32).reshape(NT, 128).T)
    shared = {
        "WqT": np.ascontiguousarray(np.asarray(Wq, np.float32).T).astype(mmdt),
        "WkT": np.ascontiguousarray(np.asarray(Wk, np.float32).T).astype(mmdt),
        "WvT": np.ascontiguousarray(np.asarray(Wv, np.float32).T).astype(mmdt),
        "WoT": np.ascontiguousarray(np.asarray(Wo, np.float32).T).astype(mmdt),
        "bv": np.asarray(bv, np.float32).astype(mmdt),
        "bo": np.asarray(bo, np.float32).astype(mmdt),
        "ones": np.ones([S], mmdt),
        "masks": masks, "ident": ident,
    }
    in_maps = []
    for b in range(B):
        ids_r = input_ids[b].astype(np.float32).reshape(NT, 128).T
        m = dict(shared)
        m["xT"] = np.ascontiguousarray(x[b].T).astype(mmdt)
        m["smalls"] = np.ascontiguousarray(
            np.concatenate([ids_r, bq_r, bk_r], axis=1)).astype(np.float32)
        in_maps.append(m)
    return in_maps


def kernel(x, input_ids, Wq, bq, Wk, bk, Wv, bv, Wo, bo):
    global LAST_RESULT, LAST_EXEC_NS
    from concourse.bass_utils import run_bass_kernel_spmd

    if "nc" not in _CACHE:
        _CACHE["nc"] = _build_graph()
    nc = _CACHE["nc"]
    in_maps = build_in_maps(x, input_ids, Wq, bq, Wk, bk, Wv, bv, Wo, bo)

    trace = os.environ.get("KERNEL_TRACE", "0") == "1" and _install_trace_hook()
    res = run_bass_kernel_spmd(nc, in_maps, core_ids=list(range(B)), trace=trace)
    LAST_RESULT = res
    LAST_EXEC_NS = res.exec_time_ns
    return np.stack([res.results[b]["out"] for b in range(B)]).astype(np.float32)
